# revision 46
# baseline (speedup 1.0000x reference)
"""Trainium2 Bass kernel for nn_FractalFieldClassifier.

Strategy (data-parallel over batch, 4 samples per core on 8 cores):
  - State z = [zr; zi] stacked on 128 SBUF partitions (64 channels x {re, im}),
    per-sample spatial layout padded to 66x66 (circular halo) so every conv tap
    is a clean strided matmul read.
  - Complex 3x3 circular conv = 9 taps; each tap is ONE 128x128 matmul
    lhsT = [[Wr, Wi], [-Wi, Wr]] accumulating into PSUM. A 10th "tap" with an
    identity selector adds the input injection from a packed inj buffer.
  - modReLU + damped update per quarter-sample (1024 px) chunk:
      sq = c^2 (ACT, PSUM->SBUF); DMA lower-half squares up; msq = cr^2+ci^2;
      mag' = sqrt(msq/a^2 + eps/a^2) = mag/a ; v = relu(a*mag' + b) = relu(mag+b)
      w = v / mag' = a*relu(mag+b)/mag ; DMA-broadcast w to both halves;
      ac = w * c ; nz = (1-a)*z + ac written in-place into the padded state.
  - Convergence stats via polarization: per step accumulate q = sum(nz^2)
    (ACT accum_out) and p' = sum(ac * z_old) (DVE scalar_tensor_tensor with
    fused accum_out); host reconstructs ||nz - z||^2 = q_k - 2*((1-a)*q_{k-1} + p'_k) + q_{k-1}.
  - Readout pooling on device; tiny dense layer + sqrt/divide on host in f64.

kernel() is self-contained: hardcoded shapes, host-side sharding across 8 cores.
"""

import math
from contextlib import ExitStack

import numpy as np

B, CCH, H, W = 32, 64, 64, 64
NC_CLASSES = 10
NCORES = 8
SPB = B // NCORES          # samples per core
NSTEPS = 30
EPS = 1e-6
HP, WP = H + 2, W + 2       # padded spatial dims (66 x 66)
NPIX = H * W                # 4096
QCHUNKS = 4                 # quarter-sample chunks per sample
QROWS = H // QCHUNKS        # 16 rows per chunk
QF = QROWS * W              # 1024 free elems per chunk
NBANK = 512                 # fp32 matmul max moving free dim / psum bank

TAPS = [(dy, dx) for dy in range(3) for dx in range(3)]


def build_nc(alpha: float, n_steps: int = NSTEPS, spb: int = SPB,
             use_f32r: bool = True, debug_dump: bool = False):
    import concourse.bacc as bacc
    import concourse.bass as bass
    import concourse.tile as tile
    from concourse import mybir

    alu = mybir.AluOpType
    act = mybir.ActivationFunctionType
    f32 = mybir.dt.float32
    f32r = mybir.dt.float32r if use_f32r else mybir.dt.float32
    a = float(alpha)

    nc = bacc.Bacc("TRN2", target_bir_lowering=False, debug=False,
                   num_devices=NCORES)

    ninj = (spb + 1) // 2
    z0pad_d = nc.dram_tensor("z0pad", [spb, CCH, HP, WP], f32, kind="ExternalInput")
    inj_d = nc.dram_tensor("inj", [ninj, 128, NPIX], f32, kind="ExternalInput")
    taps_d = nc.dram_tensor("taps", [9, 128, 128], f32, kind="ExternalInput")
    sel_d = nc.dram_tensor("sel", [2, 128, 128], f32, kind="ExternalInput")
    modb_d = nc.dram_tensor("modb", [CCH, 1], f32, kind="ExternalInput")
    pooled_d = nc.dram_tensor("pooled", [CCH, spb], f32, kind="ExternalOutput")
    qacc_d = nc.dram_tensor("qacc", [128, spb * n_steps * QCHUNKS], f32,
                            kind="ExternalOutput")
    pacc_d = nc.dram_tensor("pacc", [128, spb * n_steps * QCHUNKS], f32,
                            kind="ExternalOutput")
    dbg_d = (nc.dram_tensor("dbg", [4, 128, QF], f32, kind="ExternalOutput")
             if debug_dump else None)

    with tile.TileContext(nc) as tc, ExitStack() as ctx:
        state = ctx.enter_context(tc.tile_pool(name="state", bufs=1))
        psum = ctx.enter_context(
            tc.tile_pool(name="psum", bufs=4, space=bass.MemorySpace.PSUM))
        work = ctx.enter_context(tc.tile_pool(name="work", bufs=4))

        # ---- persistent state ----
        # Per-sample state is split into QCHUNKS row-band tiles of QROWS rows
        # plus a duplicated +-1 row halo (18 x 66 each). Separate tiles keep
        # Tile's dependency tracking chunk-local so chunks pipeline.
        # State, injection, and weights live in float32r: the PE streams f32r
        # at 1 col/cycle (vs 4 for fp32) at ~13-bit mantissa precision. All
        # producers must be compute ops (rounding); non-matmul readers bitcast
        # back to f32.
        BROWS = QROWS + 2
        zband = []  # zband[s][qt]: [128, BROWS, WP]
        for s in range(spb):
            zband.append([state.tile([128, BROWS, WP], f32r, name=f"zb{s}_{qt}")
                          for qt in range(QCHUNKS)])
        injt = []
        for j in range(ninj):
            it = state.tile([128, NPIX], f32r, name=f"inj{j}")
            injt.append(it)
        tapst = state.tile([128, 9, 128], f32r, name="tapst")
        selt = state.tile([128, 2, 128], f32r, name="selt")
        modb_hi = state.tile([128, 1], f32, name="modb_hi")
        epsb = state.tile([128, 1], f32, name="epsb")
        zerob = state.tile([128, 1], f32, name="zerob")
        qacc_t = state.tile([128, spb * n_steps * QCHUNKS], f32, name="qacc_t")
        pacc_t = state.tile([128, spb * n_steps * QCHUNKS], f32, name="pacc_t")
        pooled_t = state.tile([128, spb], f32, name="pooled_t")

        # ---- init DMAs (DMA into f32 staging, round into f32r via copy) ----
        for s in range(spb):
            for qt in range(QCHUNKS):
                zstg = work.tile([128, BROWS, WP], f32, tag="zstg", bufs=1)
                nc.sync.dma_start(
                    out=zstg[0:CCH, :, :],
                    in_=z0pad_d[s, :, qt * QROWS: qt * QROWS + BROWS, :])
                nc.vector.memset(zstg[CCH:128, :, :], 0.0)  # zi = 0
                nc.vector.tensor_copy(zband[s][qt][:, :, :], zstg[:, :, :])
        for j in range(ninj):
            for h in range(4):
                istg = work.tile([128, NPIX // 4], f32, tag="istg", bufs=1)
                sl = slice(h * NPIX // 4, (h + 1) * NPIX // 4)
                nc.sync.dma_start(out=istg[:, :], in_=inj_d[j, :, sl])
                nc.vector.tensor_copy(injt[j][:, sl], istg[:, :])
        wstg = work.tile([128, 11, 128], f32, tag="wstg", bufs=1)
        nc.sync.dma_start(out=wstg[:, 0:9, :],
                          in_=taps_d.rearrange("t k m -> k t m"))
        nc.sync.dma_start(out=wstg[:, 9:11, :],
                          in_=sel_d.rearrange("t k m -> k t m"))
        nc.vector.tensor_copy(tapst[:, :, :], wstg[:, 0:9, :])
        nc.vector.tensor_copy(selt[:, :, :], wstg[:, 9:11, :])
        nc.sync.dma_start(out=modb_hi[CCH:128, :], in_=modb_d[:, :])
        nc.sync.dma_start(out=modb_hi[0:CCH, :], in_=modb_d[:, :])
        nc.vector.memset(pacc_t[:, :], 0.0)

        inv_a2 = 1.0 / (a * a)
        nc.vector.memset(epsb[:, :], EPS * inv_a2)
        nc.vector.memset(zerob[:, :], 0.0)

        # ---- main loop (fully unrolled) ----
        def emit_matmuls(s, qt):
            """Conv taps + injection for one quarter-sample chunk -> PSUM."""
            zb = zband[s][qt]
            cps = psum.tile([128, 2, NBANK], f32, tag="cps", name=f"cps{s}_{qt}")
            for j, (dy, dx) in enumerate(TAPS):
                for u in range(2):
                    rhs = zb[:, dy + 8 * u: dy + 8 * u + 8, dx: dx + W]
                    nc.tensor.matmul(cps[:, u, :], tapst[:, j, :], rhs,
                                     start=(j == 0), stop=False)
            for u in range(2):
                ib = injt[s // 2][:, qt * QF + NBANK * u: qt * QF + NBANK * (u + 1)]
                nc.tensor.matmul(cps[:, u, :], selt[:, s % 2, :], ib,
                                 start=False, stop=True)
            return cps

        # Per-chunk working slots (4 tags, heavily reused within the chain so
        # bufs=4 keeps 4 chunks in flight within the SBUF budget):
        #   sqt:  sq -> (v @hi after msq) -> TTR junk out
        #   mvt:  DMA'd cr^2 @hi -> msq in place -> irec @hi -> q junk out
        #   magt: mag'
        #   wt:   w @hi -> DMA broadcast @lo -> ac in place (full)
        def stage_a(st):
            """sq -> DMA-up -> msq -> mag' -> v   (ACT / DMA / Pool / ACT)"""
            c = st["cps"].rearrange("p u n -> p (u n)")
            if dbg_d is not None and st["s"] == 0 and st["qt"] == 0 \
                    and st["step"] == 0:
                dbgt = work.tile([128, QF], f32, tag="dbgt", bufs=1)
                nc.scalar.activation(dbgt[:, :], c, act.Copy)
                nc.sync.dma_start(out=dbg_d[0], in_=dbgt[:, :])
            sqt = work.tile([128, QF], f32, tag="sqt")
            nc.scalar.activation(sqt[:, :], c, act.Square)
            # swap halves both ways so BOTH partition halves see the partner's
            # square; the whole scale chain then runs full-width at offset 0
            # (reciprocal_approx_fast mis-executes at base_partition=64) and
            # no separate w-broadcast is needed.
            mvt = work.tile([128, QF], f32, tag="mvt")
            nc.sync.dma_start(out=mvt[CCH:128, :], in_=sqt[0:CCH, :])
            nc.sync.dma_start(out=mvt[0:CCH, :], in_=sqt[CCH:128, :])
            nc.gpsimd.tensor_tensor(mvt[:, :], sqt[:, :], mvt[:, :], alu.add)
            magt = work.tile([128, QF], f32, tag="magt")
            nc.scalar.activation(magt[:, :], mvt[:, :],
                                 act.Sqrt, bias=epsb[:, :], scale=inv_a2)
            # v overwrites the (dead) sqt
            nc.scalar.activation(sqt[:, :], magt[:, :],
                                 act.Relu, bias=modb_hi[:, :], scale=a)
            st.update(sqt=sqt, mvt=mvt, magt=magt)

        def stage_b(st):
            """irec -> w -> ac   (DVE / Pool / DVE), all full-width"""
            c = st["cps"].rearrange("p u n -> p (u n)")
            # irec overwrites the (dead) mvt
            nc.vector.reciprocal_approx_fast(out=st["mvt"][:, :],
                                             in_=st["magt"][:, :])
            wt = work.tile([128, QF], f32, tag="wt")
            nc.gpsimd.tensor_tensor(wt[:, :], st["sqt"][:, :],
                                    st["mvt"][:, :], alu.mult)
            dbg_on = dbg_d is not None and st["s"] == 0 and st["qt"] == 0 \
                and st["step"] == 0
            if dbg_on:
                dbgt = work.tile([128, QF], f32, tag="dbgt", bufs=1)
                nc.scalar.activation(dbgt[:, :], wt[:, :], act.Copy)
                nc.sync.dma_start(out=dbg_d[1], in_=dbgt[:, :])
            # ac = w * c, in place over wt
            nc.vector.tensor_tensor(wt[:, :], wt[:, :], c, alu.mult)
            if dbg_on:
                dbgt = work.tile([128, QF], f32, tag="dbgt", bufs=1)
                nc.scalar.activation(dbgt[:, :], wt[:, :], act.Copy)
                nc.sync.dma_start(out=dbg_d[2], in_=dbgt[:, :])
            st.update(wt=wt)

        # stage_c ordering: the p'-stat needs (ac, z_old) and nz overwrites
        # z_old in place, so the stat runs first; throwaway elementwise
        # outputs land in the chunk's dead tiles. tensor_tensor_reduce is
        # broken on this hardware/runtime combo (crashes the exec unit), so
        # the stat uses scalar_tensor_tensor's fused accum_out instead, one
        # cell per chunk.
        def stage_c(st):
            """p'-stat -> nz (in place, rounds to f32r) -> q-stat."""
            s, qt, step = st["s"], st["qt"], st["step"]
            zc = zband[s][qt][:, 1: 1 + QROWS, 1: 1 + W]   # f32r center
            zcf = zc.bitcast(f32)
            ac3 = st["wt"].rearrange("p (r c) -> p r c", c=W)
            cell_i = (s * n_steps + step) * QCHUNKS + qt
            nc.vector.scalar_tensor_tensor(
                out=st["sqt"].rearrange("p (r c) -> p r c", c=W), in0=ac3,
                scalar=1.0, in1=zcf, op0=alu.mult, op1=alu.mult,
                accum_out=pacc_t[:, cell_i: cell_i + 1])
            nc.vector.scalar_tensor_tensor(
                out=zc, in0=zcf, scalar=1.0 - a, in1=ac3,
                op0=alu.mult, op1=alu.add)
            qcell_i = (s * n_steps + step) * QCHUNKS + qt
            nc.scalar.activation(st["mvt"].rearrange("p (r c) -> p r c", c=W),
                                 zcf, act.Square,
                                 accum_out=qacc_t[:, qcell_i: qcell_i + 1])

        def emit_halos(s):
            # f32r tensors need compute-op producers (DMA writes would fail the
            # FP32r rounding check), so halo refresh stays on GPSIMD copies.
            g = nc.gpsimd
            for qt in range(QCHUNKS):
                zb = zband[s][qt]
                g.tensor_copy(zb[:, 1:1 + QROWS, 0:1],
                              zb[:, 1:1 + QROWS, W:W + 1])
                g.tensor_copy(zb[:, 1:1 + QROWS, W + 1:W + 2],
                              zb[:, 1:1 + QROWS, 1:2])
            for qt in range(QCHUNKS):
                up = zband[s][(qt + 1) % QCHUNKS]
                dn = zband[s][(qt - 1) % QCHUNKS]
                g.tensor_copy(zband[s][qt][:, QROWS + 1:QROWS + 2, :],
                              up[:, 1:2, :])
                g.tensor_copy(zband[s][qt][:, 0:1, :],
                              dn[:, QROWS:QROWS + 1, :])

        # 3-stage software pipeline over the step's spb*QCHUNKS chunks, so each
        # engine's in-order queue always has the next chunk's work behind the
        # current chunk's (engines are strict FIFO — emission order is the
        # schedule).
        for step in range(n_steps):
            chunks = [{"s": s, "qt": qt, "step": step}
                      for s in range(spb) for qt in range(QCHUNKS)]
            n = len(chunks)
            for i in range(n + 2):
                if i < n:
                    chunks[i]["cps"] = emit_matmuls(chunks[i]["s"],
                                                    chunks[i]["qt"])
                    stage_a(chunks[i])
                if 1 <= i < n + 1:
                    stage_b(chunks[i - 1])
                if 2 <= i:
                    st = chunks[i - 2]
                    stage_c(st)
                    if st["qt"] == QCHUNKS - 1:
                        emit_halos(st["s"])

        # ---- readout pooling: mean over pixels of |z| (per chunk, reusing
        # the main-loop work tags) ----
        pooled4 = state.tile([128, spb * QCHUNKS], f32, name="pooled4")
        for s in range(spb):
            for qt in range(QCHUNKS):
                zc = zband[s][qt][:, 1: 1 + QROWS, 1: 1 + W].bitcast(f32)
                fsq = work.tile([128, QF], f32, tag="sqt")
                nc.scalar.activation(fsq.rearrange("p (r c) -> p r c", c=W),
                                     zc, act.Square)
                fmv = work.tile([128, QF], f32, tag="mvt")
                nc.sync.dma_start(out=fmv[CCH:128, :], in_=fsq[0:CCH, :])
                nc.vector.tensor_tensor(fmv[CCH:128, :], fsq[CCH:128, :],
                                        fmv[CCH:128, :], alu.add)
                fmag = work.tile([128, QF], f32, tag="magt")
                nc.scalar.activation(fmag[CCH:128, :], fmv[CCH:128, :],
                                     act.Sqrt, bias=zerob[CCH:128, :], scale=1.0)
                nc.vector.tensor_reduce(pooled4[CCH:128, s * QCHUNKS + qt:
                                                s * QCHUNKS + qt + 1],
                                        fmag[CCH:128, :],
                                        axis=mybir.AxisListType.X, op=alu.add)
            nc.vector.tensor_reduce(pooled_t[CCH:128, s:s + 1],
                                    pooled4[CCH:128, s * QCHUNKS:
                                            (s + 1) * QCHUNKS],
                                    axis=mybir.AxisListType.X, op=alu.add)

        # ---- output DMAs ----
        nc.sync.dma_start(out=pooled_d[:, :], in_=pooled_t[CCH:128, :])
        nc.sync.dma_start(out=qacc_d[:, :], in_=qacc_t[:, :])
        nc.sync.dma_start(out=pacc_d[:, :], in_=pacc_t[:, :])

    nc.compile()
    return nc


def _host_prep(x, enc_w, enc_b, kern_wr, kern_wi):
    """Spectral norm + encoder conv + padded initial state, all on host."""
    x = np.asarray(x, np.float32)
    C = kern_wr.shape[0]
    Wc = (np.asarray(kern_wr, np.float64) + 1j * np.asarray(kern_wi, np.float64)
          ).reshape(C, -1)
    sigma = np.linalg.norm(Wc, ord=2)
    wr = (np.asarray(kern_wr, np.float64) / sigma).astype(np.float32)
    wi = (np.asarray(kern_wi, np.float64) / sigma).astype(np.float32)

    xp = np.pad(x[:, 0], ((0, 0), (1, 1), (1, 1)), mode="wrap")
    cols = np.empty((9, B, H, W), np.float32)
    for k, (dy, dx) in enumerate(TAPS):
        cols[k] = xp[:, dy:dy + H, dx:dx + W]
    w9 = np.asarray(enc_w, np.float32).reshape(CCH, 9)
    zr0 = np.einsum("ok,kbhw->bohw", w9, cols, optimize=True) \
        + np.asarray(enc_b, np.float32)[None, :, None, None]
    zr0 = zr0.astype(np.float32)
    z0pad = np.pad(zr0, ((0, 0), (0, 0), (1, 1), (1, 1)), mode="wrap")
    return wr, wi, zr0, z0pad


def _tap_matrices(wr, wi):
    taps = np.zeros((9, 128, 128), np.float32)
    for j, (dy, dx) in enumerate(TAPS):
        Wr = wr[:, :, dy, dx]   # [out, in]
        Wi = wi[:, :, dy, dx]
        taps[j, 0:CCH, 0:CCH] = Wr.T          # lhsT[k=in, m=out]
        taps[j, CCH:128, 0:CCH] = -Wi.T
        taps[j, 0:CCH, CCH:128] = Wi.T
        taps[j, CCH:128, CCH:128] = Wr.T
    sel = np.zeros((2, 128, 128), np.float32)
    sel[0, 0:CCH, 0:CCH] = np.eye(CCH, dtype=np.float32)
    sel[1, CCH:128, 0:CCH] = np.eye(CCH, dtype=np.float32)
    return taps, sel


_NC_CACHE = {}
LAST_RESULTS = None  # BassKernelResults of the most recent kernel() call


def _get_nc(alpha: float):
    key = round(float(alpha), 10)
    if key not in _NC_CACHE:
        _NC_CACHE[key] = build_nc(key)
    return _NC_CACHE[key]


def make_in_map(z0pad_slice, zr0_slice, taps, sel, modb, spb=SPB):
    inj = np.zeros(((spb + 1) // 2, 128, NPIX), np.float32)
    for j in range((spb + 1) // 2):
        inj[j, 0:CCH] = zr0_slice[2 * j].reshape(CCH, NPIX)
        if 2 * j + 1 < spb:
            inj[j, CCH:128] = zr0_slice[2 * j + 1].reshape(CCH, NPIX)
    return {
        "z0pad": np.ascontiguousarray(z0pad_slice),
        "inj": inj,
        "taps": taps,
        "sel": sel,
        "modb": modb,
    }


def kernel(x, enc_w, enc_b, kern_wr, kern_wi, mod_b, alpha, ro_w, ro_b):
    from concourse.bass_utils import run_bass_kernel_spmd

    a = float(np.asarray(alpha))
    wr, wi, zr0, z0pad = _host_prep(x, enc_w, enc_b, kern_wr, kern_wi)
    taps, sel = _tap_matrices(wr, wi)
    modb = np.asarray(mod_b, np.float32).reshape(CCH, 1)

    in_maps = []
    for c in range(NCORES):
        s0 = c * SPB
        in_maps.append(make_in_map(z0pad[s0:s0 + SPB], zr0[s0:s0 + SPB],
                                   taps, sel, modb))

    nc = _get_nc(a)
    res = run_bass_kernel_spmd(nc, in_maps, core_ids=list(range(NCORES)))
    global LAST_RESULTS
    LAST_RESULTS = res

    # ---- host-side gather / finalization (tiny, f64) ----
    pooled = np.zeros((B, CCH), np.float64)
    Q = np.zeros(NSTEPS + 1, np.float64)      # Q[k] = sum(z_k^2) global
    Pp = np.zeros(NSTEPS, np.float64)         # sum(ac * z_old) per step
    Q[0] = float(np.sum(zr0.astype(np.float64) ** 2))
    for c in range(NCORES):
        out = res.results[c]
        pooled[c * SPB:(c + 1) * SPB] = out["pooled"].T / float(NPIX)
        qa = out["qacc"].astype(np.float64).sum(axis=0)  # [spb*NSTEPS*QCHUNKS]
        qa = qa.reshape(SPB, NSTEPS, QCHUNKS).sum(axis=(0, 2))
        Q[1:] += qa
        pa = out["pacc"].astype(np.float64).sum(axis=0)
        Pp += pa.reshape(SPB, NSTEPS, QCHUNKS).sum(axis=(0, 2))

    history = np.zeros(NSTEPS, np.float64)
    for k in range(NSTEPS):
        p_k = (1.0 - a) * Q[k] + Pp[k]
        dz2 = Q[k + 1] - 2.0 * p_k + Q[k]
        dz2 = max(dz2, 0.0)
        nrm = math.sqrt(Q[k]) + EPS
        history[k] = math.sqrt(dz2) / nrm

    logits = pooled @ np.asarray(ro_w, np.float64).T + np.asarray(ro_b, np.float64)
    return logits.astype(np.float32), history.astype(np.float32)


# revision 51
# speedup vs baseline: 1.2239x; 1.2239x over previous
"""Trainium2 Bass kernel for nn_FractalFieldClassifier.

Strategy (data-parallel over batch, 4 samples per core on 8 cores):
  - State z = [zr; zi] stacked on 128 SBUF partitions (64 channels x {re, im}),
    per-sample spatial layout padded to 66x66 (circular halo) so every conv tap
    is a clean strided matmul read.
  - Complex 3x3 circular conv = 9 taps; each tap is ONE 128x128 matmul
    lhsT = [[Wr, Wi], [-Wi, Wr]] accumulating into PSUM. A 10th "tap" with an
    identity selector adds the input injection from a packed inj buffer.
  - modReLU + damped update per quarter-sample (1024 px) chunk:
      sq = c^2 (ACT, PSUM->SBUF); DMA lower-half squares up; msq = cr^2+ci^2;
      mag' = sqrt(msq/a^2 + eps/a^2) = mag/a ; v = relu(a*mag' + b) = relu(mag+b)
      w = v / mag' = a*relu(mag+b)/mag ; DMA-broadcast w to both halves;
      ac = w * c ; nz = (1-a)*z + ac written in-place into the padded state.
  - Convergence stats via polarization: per step accumulate q = sum(nz^2)
    (ACT accum_out) and p' = sum(ac * z_old) (DVE scalar_tensor_tensor with
    fused accum_out); host reconstructs ||nz - z||^2 = q_k - 2*((1-a)*q_{k-1} + p'_k) + q_{k-1}.
  - Readout pooling on device; tiny dense layer + sqrt/divide on host in f64.

kernel() is self-contained: hardcoded shapes, host-side sharding across 8 cores.
"""

import math
from contextlib import ExitStack

import numpy as np

B, CCH, H, W = 32, 64, 64, 64
NC_CLASSES = 10
NCORES = 8
SPB = B // NCORES          # samples per core
NSTEPS = 30
EPS = 1e-6
HP, WP = H + 2, W + 2       # padded spatial dims (66 x 66)
NPIX = H * W                # 4096
QCHUNKS = 4                 # quarter-sample chunks per sample
QROWS = H // QCHUNKS        # 16 rows per chunk
QF = QROWS * W              # 1024 free elems per chunk
NBANK = 512                 # fp32 matmul max moving free dim / psum bank

TAPS = [(dy, dx) for dy in range(3) for dx in range(3)]


def build_nc(alpha: float, n_steps: int = NSTEPS, spb: int = SPB,
             use_f32r: bool = True, debug_dump: bool = False):
    import concourse.bacc as bacc
    import concourse.bass as bass
    import concourse.tile as tile
    from concourse import mybir

    alu = mybir.AluOpType
    act = mybir.ActivationFunctionType
    f32 = mybir.dt.float32
    f32r = mybir.dt.float32r if use_f32r else mybir.dt.float32
    a = float(alpha)

    nc = bacc.Bacc("TRN2", target_bir_lowering=False, debug=False,
                   num_devices=NCORES)

    ninj = (spb + 1) // 2
    z0pad_d = nc.dram_tensor("z0pad", [spb, CCH, HP, WP], f32, kind="ExternalInput")
    taps_d = nc.dram_tensor("taps", [9, 128, 128], f32, kind="ExternalInput")
    sel_d = nc.dram_tensor("sel", [2, 128, 128], f32, kind="ExternalInput")
    modb_d = nc.dram_tensor("modb", [CCH, 1], f32, kind="ExternalInput")
    pooled_d = nc.dram_tensor("pooled", [CCH, spb], f32, kind="ExternalOutput")
    qacc_d = nc.dram_tensor("qacc", [128, spb * n_steps * QCHUNKS], f32,
                            kind="ExternalOutput")
    pacc_d = nc.dram_tensor("pacc", [128, spb * n_steps * QCHUNKS], f32,
                            kind="ExternalOutput")
    dbg_d = (nc.dram_tensor("dbg", [4, 128, QF], f32, kind="ExternalOutput")
             if debug_dump else None)

    with tile.TileContext(nc) as tc, ExitStack() as ctx:
        state = ctx.enter_context(tc.tile_pool(name="state", bufs=1))
        psum = ctx.enter_context(
            tc.tile_pool(name="psum", bufs=4, space=bass.MemorySpace.PSUM))
        work = ctx.enter_context(tc.tile_pool(name="work", bufs=4))

        # ---- persistent state ----
        # Per-sample state is split into QCHUNKS row-band tiles of QROWS rows
        # plus a duplicated +-1 row halo (18 x 66 each). Separate tiles keep
        # Tile's dependency tracking chunk-local so chunks pipeline.
        # State, injection, and weights live in float32r: the PE streams f32r
        # at 1 col/cycle (vs 4 for fp32) at ~13-bit mantissa precision. All
        # producers must be compute ops (rounding); non-matmul readers bitcast
        # back to f32.
        BROWS = QROWS + 2
        zband = []  # zband[s][qt]: [128, BROWS, WP]
        for s in range(spb):
            zband.append([state.tile([128, BROWS, WP], f32r, name=f"zb{s}_{qt}")
                          for qt in range(QCHUNKS)])
        injt = []
        for j in range(ninj):
            it = state.tile([128, NPIX], f32r, name=f"inj{j}")
            injt.append(it)
        tapst = state.tile([128, 9, 128], f32r, name="tapst")
        selt = state.tile([128, 2, 128], f32r, name="selt")
        modb_hi = state.tile([128, 1], f32, name="modb_hi")
        epsb = state.tile([128, 1], f32, name="epsb")
        zerob = state.tile([128, 1], f32, name="zerob")
        qacc_t = state.tile([128, spb * n_steps * QCHUNKS], f32, name="qacc_t")
        pacc_t = state.tile([128, spb * n_steps * QCHUNKS], f32, name="pacc_t")
        pooled_t = state.tile([128, spb], f32, name="pooled_t")

        # ---- init DMAs (DMA into f32 staging, round into f32r via copy) ----
        for s in range(spb):
            for qt in range(QCHUNKS):
                zstg = work.tile([128, BROWS, WP], f32, tag="zstg", bufs=1)
                nc.sync.dma_start(
                    out=zstg[0:CCH, :, :],
                    in_=z0pad_d[s, :, qt * QROWS: qt * QROWS + BROWS, :])
                nc.vector.memset(zstg[CCH:128, :, :], 0.0)  # zi = 0
                nc.vector.tensor_copy(zband[s][qt][:, :, :], zstg[:, :, :])
        # The injection buffer duplicates z0 (packed two samples per buffer),
        # so build it on-device from the freshly loaded state instead of
        # uploading another 4 MB: DMA gathers both samples' zr0 into an f32
        # staging tile (cross-partition moves are DMA-only), then a copy
        # rounds into f32r.
        for j in range(ninj):
            for qt in range(QCHUNKS):
                istg = work.tile([128, QROWS, W], f32, tag="istg", bufs=1)
                srcA = zband[2 * j][qt][0:CCH, 1:1 + QROWS, 1:1 + W].bitcast(f32)
                nc.sync.dma_start(out=istg[0:CCH, :, :], in_=srcA)
                sB = min(2 * j + 1, spb - 1)
                srcB = zband[sB][qt][0:CCH, 1:1 + QROWS, 1:1 + W].bitcast(f32)
                nc.sync.dma_start(out=istg[CCH:128, :, :], in_=srcB)
                nc.vector.tensor_copy(
                    injt[j][:, qt * QF:(qt + 1) * QF].rearrange(
                        "p (r c) -> p r c", c=W), istg[:, :, :])
        wstg = work.tile([128, 11, 128], f32, tag="wstg", bufs=1)
        nc.sync.dma_start(out=wstg[:, 0:9, :],
                          in_=taps_d.rearrange("t k m -> k t m"))
        nc.sync.dma_start(out=wstg[:, 9:11, :],
                          in_=sel_d.rearrange("t k m -> k t m"))
        nc.vector.tensor_copy(tapst[:, :, :], wstg[:, 0:9, :])
        nc.vector.tensor_copy(selt[:, :, :], wstg[:, 9:11, :])
        nc.sync.dma_start(out=modb_hi[CCH:128, :], in_=modb_d[:, :])
        nc.sync.dma_start(out=modb_hi[0:CCH, :], in_=modb_d[:, :])
        nc.vector.memset(pacc_t[:, :], 0.0)

        inv_a2 = 1.0 / (a * a)
        nc.vector.memset(epsb[:, :], EPS * inv_a2)
        nc.vector.memset(zerob[:, :], 0.0)

        # ---- main loop (fully unrolled) ----
        def emit_matmuls(s, qt):
            """Conv taps + injection for one quarter-sample chunk -> PSUM."""
            zb = zband[s][qt]
            cps = psum.tile([128, 2, NBANK], f32, tag="cps", name=f"cps{s}_{qt}")
            for j, (dy, dx) in enumerate(TAPS):
                for u in range(2):
                    rhs = zb[:, dy + 8 * u: dy + 8 * u + 8, dx: dx + W]
                    nc.tensor.matmul(cps[:, u, :], tapst[:, j, :], rhs,
                                     start=(j == 0), stop=False)
            for u in range(2):
                ib = injt[s // 2][:, qt * QF + NBANK * u: qt * QF + NBANK * (u + 1)]
                nc.tensor.matmul(cps[:, u, :], selt[:, s % 2, :], ib,
                                 start=False, stop=True)
            return cps

        # Per-chunk working slots (4 tags, heavily reused within the chain so
        # bufs=4 keeps 4 chunks in flight within the SBUF budget):
        #   sqt:  sq -> (v @hi after msq) -> TTR junk out
        #   mvt:  DMA'd cr^2 @hi -> msq in place -> irec @hi -> q junk out
        #   magt: mag'
        #   wt:   w @hi -> DMA broadcast @lo -> ac in place (full)
        def stage_a(st):
            """sq -> DMA-up -> msq -> mag' -> v   (ACT / DMA / Pool / ACT)"""
            c = st["cps"].rearrange("p u n -> p (u n)")
            if dbg_d is not None and st["s"] == 0 and st["qt"] == 0 \
                    and st["step"] == 0:
                dbgt = work.tile([128, QF], f32, tag="dbgt", bufs=1)
                nc.scalar.activation(dbgt[:, :], c, act.Copy)
                nc.sync.dma_start(out=dbg_d[0], in_=dbgt[:, :])
            sqt = work.tile([128, QF], f32, tag="sqt")
            nc.scalar.activation(sqt[:, :], c, act.Square)
            # swap halves both ways so BOTH partition halves see the partner's
            # square; the whole scale chain then runs full-width at offset 0
            # (reciprocal_approx_fast mis-executes at base_partition=64) and
            # no separate w-broadcast is needed.
            mvt = work.tile([128, QF], f32, tag="mvt")
            nc.sync.dma_start(out=mvt[CCH:128, :], in_=sqt[0:CCH, :])
            nc.sync.dma_start(out=mvt[0:CCH, :], in_=sqt[CCH:128, :])
            nc.gpsimd.tensor_tensor(mvt[:, :], sqt[:, :], mvt[:, :], alu.add)
            magt = work.tile([128, QF], f32, tag="magt")
            nc.scalar.activation(magt[:, :], mvt[:, :],
                                 act.Sqrt, bias=epsb[:, :], scale=inv_a2)
            # v overwrites the (dead) sqt
            nc.scalar.activation(sqt[:, :], magt[:, :],
                                 act.Relu, bias=modb_hi[:, :], scale=a)
            st.update(sqt=sqt, mvt=mvt, magt=magt)

        def stage_b(st):
            """irec -> w -> ac   (DVE / Pool / DVE), all full-width"""
            c = st["cps"].rearrange("p u n -> p (u n)")
            # irec overwrites the (dead) mvt
            nc.vector.reciprocal_approx_fast(out=st["mvt"][:, :],
                                             in_=st["magt"][:, :])
            wt = work.tile([128, QF], f32, tag="wt")
            nc.gpsimd.tensor_tensor(wt[:, :], st["sqt"][:, :],
                                    st["mvt"][:, :], alu.mult)
            dbg_on = dbg_d is not None and st["s"] == 0 and st["qt"] == 0 \
                and st["step"] == 0
            if dbg_on:
                dbgt = work.tile([128, QF], f32, tag="dbgt", bufs=1)
                nc.scalar.activation(dbgt[:, :], wt[:, :], act.Copy)
                nc.sync.dma_start(out=dbg_d[1], in_=dbgt[:, :])
            # ac = w * c, in place over wt
            nc.vector.tensor_tensor(wt[:, :], wt[:, :], c, alu.mult)
            if dbg_on:
                dbgt = work.tile([128, QF], f32, tag="dbgt", bufs=1)
                nc.scalar.activation(dbgt[:, :], wt[:, :], act.Copy)
                nc.sync.dma_start(out=dbg_d[2], in_=dbgt[:, :])
            st.update(wt=wt)

        # stage_c ordering: the p'-stat needs (ac, z_old) and nz overwrites
        # z_old in place, so the stat runs first; throwaway elementwise
        # outputs land in the chunk's dead tiles. tensor_tensor_reduce is
        # broken on this hardware/runtime combo (crashes the exec unit), so
        # the stat uses scalar_tensor_tensor's fused accum_out instead, one
        # cell per chunk.
        def stage_c(st):
            """p'-stat -> nz (in place, rounds to f32r) -> q-stat."""
            s, qt, step = st["s"], st["qt"], st["step"]
            zc = zband[s][qt][:, 1: 1 + QROWS, 1: 1 + W]   # f32r center
            zcf = zc.bitcast(f32)
            ac3 = st["wt"].rearrange("p (r c) -> p r c", c=W)
            cell_i = (s * n_steps + step) * QCHUNKS + qt
            nc.vector.scalar_tensor_tensor(
                out=st["sqt"].rearrange("p (r c) -> p r c", c=W), in0=ac3,
                scalar=1.0, in1=zcf, op0=alu.mult, op1=alu.mult,
                accum_out=pacc_t[:, cell_i: cell_i + 1])
            nc.vector.scalar_tensor_tensor(
                out=zc, in0=zcf, scalar=1.0 - a, in1=ac3,
                op0=alu.mult, op1=alu.add)
            qcell_i = (s * n_steps + step) * QCHUNKS + qt
            nc.scalar.activation(st["mvt"].rearrange("p (r c) -> p r c", c=W),
                                 zcf, act.Square,
                                 accum_out=qacc_t[:, qcell_i: qcell_i + 1])

        def emit_halos(s):
            # f32r tensors need compute-op producers (DMA writes would fail the
            # FP32r rounding check), so halo refresh stays on GPSIMD copies.
            g = nc.gpsimd
            for qt in range(QCHUNKS):
                zb = zband[s][qt]
                g.tensor_copy(zb[:, 1:1 + QROWS, 0:1],
                              zb[:, 1:1 + QROWS, W:W + 1])
                g.tensor_copy(zb[:, 1:1 + QROWS, W + 1:W + 2],
                              zb[:, 1:1 + QROWS, 1:2])
            for qt in range(QCHUNKS):
                up = zband[s][(qt + 1) % QCHUNKS]
                dn = zband[s][(qt - 1) % QCHUNKS]
                g.tensor_copy(zband[s][qt][:, QROWS + 1:QROWS + 2, :],
                              up[:, 1:2, :])
                g.tensor_copy(zband[s][qt][:, 0:1, :],
                              dn[:, QROWS:QROWS + 1, :])

        # 3-stage software pipeline over the step's spb*QCHUNKS chunks, so each
        # engine's in-order queue always has the next chunk's work behind the
        # current chunk's (engines are strict FIFO — emission order is the
        # schedule).
        # One flat 3-stage software pipeline over ALL (step, sample, chunk)
        # triples — no per-step drain; step k+1's matmuls sit in the PE queue
        # right behind step k's and start as soon as that sample's halo
        # refresh lands.
        chunks = [{"s": s, "qt": qt, "step": step}
                  for step in range(n_steps)
                  for s in range(spb) for qt in range(QCHUNKS)]
        n = len(chunks)
        for i in range(n + 2):
            if i < n:
                chunks[i]["cps"] = emit_matmuls(chunks[i]["s"],
                                                chunks[i]["qt"])
                stage_a(chunks[i])
            if 1 <= i < n + 1:
                stage_b(chunks[i - 1])
            if 2 <= i:
                st = chunks[i - 2]
                stage_c(st)
                if st["qt"] == QCHUNKS - 1:
                    emit_halos(st["s"])
                st.clear()  # drop tile refs once the chunk retires

        # ---- readout pooling: mean over pixels of |z| (per chunk, reusing
        # the main-loop work tags) ----
        pooled4 = state.tile([128, spb * QCHUNKS], f32, name="pooled4")
        for s in range(spb):
            for qt in range(QCHUNKS):
                zc = zband[s][qt][:, 1: 1 + QROWS, 1: 1 + W].bitcast(f32)
                fsq = work.tile([128, QF], f32, tag="sqt")
                nc.scalar.activation(fsq.rearrange("p (r c) -> p r c", c=W),
                                     zc, act.Square)
                fmv = work.tile([128, QF], f32, tag="mvt")
                nc.sync.dma_start(out=fmv[CCH:128, :], in_=fsq[0:CCH, :])
                nc.vector.tensor_tensor(fmv[CCH:128, :], fsq[CCH:128, :],
                                        fmv[CCH:128, :], alu.add)
                fmag = work.tile([128, QF], f32, tag="magt")
                nc.scalar.activation(fmag[CCH:128, :], fmv[CCH:128, :],
                                     act.Sqrt, bias=zerob[CCH:128, :], scale=1.0)
                nc.vector.tensor_reduce(pooled4[CCH:128, s * QCHUNKS + qt:
                                                s * QCHUNKS + qt + 1],
                                        fmag[CCH:128, :],
                                        axis=mybir.AxisListType.X, op=alu.add)
            nc.vector.tensor_reduce(pooled_t[CCH:128, s:s + 1],
                                    pooled4[CCH:128, s * QCHUNKS:
                                            (s + 1) * QCHUNKS],
                                    axis=mybir.AxisListType.X, op=alu.add)

        # ---- output DMAs ----
        nc.sync.dma_start(out=pooled_d[:, :], in_=pooled_t[CCH:128, :])
        nc.sync.dma_start(out=qacc_d[:, :], in_=qacc_t[:, :])
        nc.sync.dma_start(out=pacc_d[:, :], in_=pacc_t[:, :])

    nc.compile()
    return nc


def _host_prep(x, enc_w, enc_b, kern_wr, kern_wi):
    """Spectral norm + encoder conv + padded initial state, all on host."""
    x = np.asarray(x, np.float32)
    C = kern_wr.shape[0]
    Wc = (np.asarray(kern_wr, np.float64) + 1j * np.asarray(kern_wi, np.float64)
          ).reshape(C, -1)
    sigma = np.linalg.norm(Wc, ord=2)
    wr = (np.asarray(kern_wr, np.float64) / sigma).astype(np.float32)
    wi = (np.asarray(kern_wi, np.float64) / sigma).astype(np.float32)

    xp = np.pad(x[:, 0], ((0, 0), (1, 1), (1, 1)), mode="wrap")
    cols = np.empty((9, B, H, W), np.float32)
    for k, (dy, dx) in enumerate(TAPS):
        cols[k] = xp[:, dy:dy + H, dx:dx + W]
    w9 = np.asarray(enc_w, np.float32).reshape(CCH, 9)
    zr0 = np.einsum("ok,kbhw->bohw", w9, cols, optimize=True) \
        + np.asarray(enc_b, np.float32)[None, :, None, None]
    zr0 = zr0.astype(np.float32)
    z0pad = np.pad(zr0, ((0, 0), (0, 0), (1, 1), (1, 1)), mode="wrap")
    return wr, wi, zr0, z0pad


def _tap_matrices(wr, wi):
    taps = np.zeros((9, 128, 128), np.float32)
    for j, (dy, dx) in enumerate(TAPS):
        Wr = wr[:, :, dy, dx]   # [out, in]
        Wi = wi[:, :, dy, dx]
        taps[j, 0:CCH, 0:CCH] = Wr.T          # lhsT[k=in, m=out]
        taps[j, CCH:128, 0:CCH] = -Wi.T
        taps[j, 0:CCH, CCH:128] = Wi.T
        taps[j, CCH:128, CCH:128] = Wr.T
    sel = np.zeros((2, 128, 128), np.float32)
    sel[0, 0:CCH, 0:CCH] = np.eye(CCH, dtype=np.float32)
    sel[1, CCH:128, 0:CCH] = np.eye(CCH, dtype=np.float32)
    return taps, sel


_NC_CACHE = {}
LAST_RESULTS = None  # BassKernelResults of the most recent kernel() call


def _get_nc(alpha: float):
    key = round(float(alpha), 10)
    if key not in _NC_CACHE:
        _NC_CACHE[key] = build_nc(key)
    return _NC_CACHE[key]


def make_in_map(z0pad_slice, zr0_slice, taps, sel, modb, spb=SPB):
    return {
        "z0pad": np.ascontiguousarray(z0pad_slice),
        "taps": taps,
        "sel": sel,
        "modb": modb,
    }


def kernel(x, enc_w, enc_b, kern_wr, kern_wi, mod_b, alpha, ro_w, ro_b):
    from concourse.bass_utils import run_bass_kernel_spmd

    a = float(np.asarray(alpha))
    wr, wi, zr0, z0pad = _host_prep(x, enc_w, enc_b, kern_wr, kern_wi)
    taps, sel = _tap_matrices(wr, wi)
    modb = np.asarray(mod_b, np.float32).reshape(CCH, 1)

    in_maps = []
    for c in range(NCORES):
        s0 = c * SPB
        in_maps.append(make_in_map(z0pad[s0:s0 + SPB], zr0[s0:s0 + SPB],
                                   taps, sel, modb))

    nc = _get_nc(a)
    res = run_bass_kernel_spmd(nc, in_maps, core_ids=list(range(NCORES)))
    global LAST_RESULTS
    LAST_RESULTS = res

    # ---- host-side gather / finalization (tiny, f64) ----
    pooled = np.zeros((B, CCH), np.float64)
    Q = np.zeros(NSTEPS + 1, np.float64)      # Q[k] = sum(z_k^2) global
    Pp = np.zeros(NSTEPS, np.float64)         # sum(ac * z_old) per step
    Q[0] = float(np.sum(zr0.astype(np.float64) ** 2))
    for c in range(NCORES):
        out = res.results[c]
        pooled[c * SPB:(c + 1) * SPB] = out["pooled"].T / float(NPIX)
        qa = out["qacc"].astype(np.float64).sum(axis=0)  # [spb*NSTEPS*QCHUNKS]
        qa = qa.reshape(SPB, NSTEPS, QCHUNKS).sum(axis=(0, 2))
        Q[1:] += qa
        pa = out["pacc"].astype(np.float64).sum(axis=0)
        Pp += pa.reshape(SPB, NSTEPS, QCHUNKS).sum(axis=(0, 2))

    history = np.zeros(NSTEPS, np.float64)
    for k in range(NSTEPS):
        p_k = (1.0 - a) * Q[k] + Pp[k]
        dz2 = Q[k + 1] - 2.0 * p_k + Q[k]
        dz2 = max(dz2, 0.0)
        nrm = math.sqrt(Q[k]) + EPS
        history[k] = math.sqrt(dz2) / nrm

    logits = pooled @ np.asarray(ro_w, np.float64).T + np.asarray(ro_b, np.float64)
    return logits.astype(np.float32), history.astype(np.float32)
